# revision 1
# baseline (speedup 1.0000x reference)
"""Trainium2 Bass kernel for nn_Discriminator_59442347376701.

Embedding lookup (one-hot matmul rewritten as a DMA gather) + bidirectional
LSTM + small MLP head, distributed over 8 NeuronCores as
(direction x batch-quarter).  Core c: direction = c//4 (0=fwd, 1=rev),
batch quarter g = c%4 (global sequences g*8 .. g*8+8).  Reverse cores get
time-reversed token indices from the host so the device program is uniform
SPMD; sequence lengths / last-position latching are computed on device.

Layout: everything transposed -- hidden/gate dims on partitions, batch on
the free dim.  The scan is latency-bound (~2us per step through
PE -> sigmoid -> cell update -> tanh -> h), so each core runs one chain of
its 8 sequences; recurrent weights are fp8e4m3 (halves the per-step
LDWEIGHTS stream), activations bf16, cell state fp32.  Gates are grouped
{f,g} / {i,o} in separate PSUM tiles so sigmoid(f) / tanh(g) / f*c hide
under the {i,o} matmuls.

The head needs h_fwd and h_rev together: each pair {g, g+4} combines
partial W1 products with one small AllReduce, then forward core g emits
sigmoid(head) for its 8 sequences; the host concatenates 4x[8].
"""
import os
import sys

sys.path.insert(0, "/opt/trn_rl_repo")

import contextlib
import numpy as np
import ml_dtypes

import concourse.bass as bass
import concourse.tile as tile
from concourse import bacc, mybir
from concourse.bass_utils import run_bass_kernel_spmd

F32 = mybir.dt.float32
BF16 = mybir.dt.bfloat16
I32 = mybir.dt.int32
AF = mybir.ActivationFunctionType
ALU = mybir.AluOpType

VOCAB, EMB, H, LATENT, B, S = 50257, 128, 256, 64, 32, 128
G4 = 4 * H          # 1024 gate dims
NC = 8              # cores
BC = 8              # sequences per core
TOK = BC * S        # 1024 tokens per core
DBG = bool(int(os.environ.get("KDBG", "0")))
# Phase bisection for perf debugging: 1=gather, 2=+transpose/xg, 3=+scan, 4=full
PHASE = int(os.environ.get("KPHASE", "4"))
FP8 = bool(int(os.environ.get("KFP8", "1")))  # fp8e4m3 recurrent weights
BF16NP = ml_dtypes.bfloat16
FP8NP = ml_dtypes.float8_e4m3
WHH_DT = mybir.dt.float8e4 if FP8 else mybir.dt.bfloat16
WHH_NP = FP8NP if FP8 else BF16NP


def _ap(base, layout):
    """Hand-built access pattern (for stride-0 broadcasts / reordered dims)."""
    return bass.AP(base.tensor, base.offset, layout)


def _emit(nc, tc, d):
    ctx = contextlib.ExitStack()
    with ctx:
        const = ctx.enter_context(tc.tile_pool(name="const", bufs=1))
        big = ctx.enter_context(tc.tile_pool(name="big", bufs=1))
        work = ctx.enter_context(tc.tile_pool(name="work", bufs=4))
        scan = ctx.enter_context(tc.tile_pool(name="scan", bufs=6))
        ps_scan = ctx.enter_context(tc.tile_pool(name="ps_scan", bufs=4, space="PSUM"))
        ps_xg = ctx.enter_context(tc.tile_pool(name="ps_xg", bufs=2, space="PSUM"))
        ps_tr = ctx.enter_context(tc.tile_pool(name="ps_tr", bufs=1, space="PSUM"))

        def load(name, shape, dt):
            t = const.tile(list(shape), dt, tag=name)
            nc.sync.dma_start(t[:], d[name][:])
            return t

        idx = load("idx", (128, BC), I32)
        idxa = load("idxa", (BC, 1), I32)
        whhT = load("whhT", (128, 2 * G4), WHH_DT)
        wihT = load("wihT", (128, G4), BF16)
        bvec = load("bvec", (128, 8), F32)
        idf = load("identf", (128, 128), F32)
        idb = load("identb", (128, 128), BF16)
        onesb = load("onesb", (128, 128), BF16)
        c0c1 = load("c0c1", (128, 2), F32)
        w1ta = load("W1TA", (128, 512), BF16)
        w1tb = load("W1TB", (128, 256), BF16)
        w2t = load("W2T", (128, 128), BF16)
        wdt = load("WdT", (64, 1), BF16)
        b1c = load("b1c", (128, 2), F32)
        b2c = load("b2c", (64, 1), F32)
        bdc = load("bdc", (1, 1), F32)
        al0 = load("al0", (128, 1), F32)
        al1 = load("al1", (128, 1), F32)

        # First ACT instruction is a sigmoid so the table chooser settles on
        # sigmoid_and_others (contains sigmoid/tanh/identity/copy) -- avoids
        # a second ~2.7us ACT_TABLE_LOAD right at scan start.
        actwarm = const.tile([1, 1], F32, tag="actwarm", name="actwarm")
        nc.scalar.activation(actwarm[:], al0[0:1, 0:1], AF.Sigmoid)

        def finish_stub():
            ostub = const.tile([1, BC], F32, tag="outs_stub", name="outs_stub")
            nc.vector.memset(ostub[:], 0.5)
            nc.sync.dma_start(d["out"][:], ostub[:])

        if PHASE < 1:
            finish_stub()
            return

        # ---- embedding gather: token n = m*128+p -> g_nat[p, m*128:(m+1)*128] ----
        g_nat = big.tile([128, TOK], F32, tag="g_nat")
        for q in range(4):
            nc.gpsimd.indirect_dma_start(
                out=g_nat[:, q * 256:(q + 1) * 256], out_offset=None,
                in_=d["W_emb"][:],
                in_offset=bass.IndirectOffsetOnAxis(ap=idx[:, q * 2:(q + 1) * 2], axis=0))
        g_a = work.tile([BC, 128], F32, tag="g_a")
        nc.gpsimd.indirect_dma_start(
            out=g_a[:], out_offset=None,
            in_=d["W_emb"][:],
            in_offset=bass.IndirectOffsetOnAxis(ap=idxa[:], axis=0))

        if PHASE < 2:
            finish_stub()
            return

        # ---- transpose blocks -> embT [128, 1024] bf16, col n = t*8 + j ----
        embT = big.tile([128, TOK], BF16, tag="embT")
        for m in range(8):
            pt = ps_tr.tile([128, 128], F32, tag="ps_tr")
            nc.tensor.transpose(pt[:], g_nat[:, m * 128:(m + 1) * 128], idf[:])
            if m % 2 == 0:
                nc.vector.tensor_copy(embT[:, m * 128:(m + 1) * 128], pt[:])
            else:
                nc.scalar.copy(embT[:, m * 128:(m + 1) * 128], pt[:])

        pa = ps_tr.tile([128, 128], F32, tag="ps_tr")
        nc.tensor.transpose(pa[:, 0:BC], g_a[:], idf[0:BC, 0:BC])
        embaT = const.tile([128, BC], F32, tag="embaT")
        nc.vector.tensor_copy(embaT[:], pa[:, 0:BC])

        # ---- xg projection -> xg [128, S*64] bf16; col = t*64 + mc*8 + b ----
        xgh = [big.tile([128, S * 32], BF16, tag="xgA", name="xgA"),
               big.tile([128, S * 32], BF16, tag="xgB", name="xgB")]
        for half in range(2):
            for mc in range(8):
                pxg = ps_xg.tile([128, 512], F32, tag="ps_xg")
                nc.tensor.matmul(
                    pxg[:], lhsT=wihT[:, mc * 128:(mc + 1) * 128],
                    rhs=embT[:, half * 512:(half + 1) * 512],
                    start=True, stop=True)
                rd = pxg[:].rearrange("p (t j) -> p t j", j=8)
                wr = _ap(xgh[half][:, mc * 8],
                         [[S * 32, 128], [64, 64], [1, 8]])
                bmc = bvec[:, mc:mc + 1]
                if mc % 2 == 0:
                    nc.vector.tensor_scalar(wr, rd, bmc, None, op0=ALU.add)
                else:
                    nc.scalar.activation(wr, rd, AF.Identity, bias=bmc, scale=1.0)

        if PHASE < 3:
            finish_stub()
            return

        # ---- lengths + latch masks ----
        nz = work.tile([128, BC], BF16, tag="nz")
        nc.vector.tensor_scalar(nz[:], idx[:], 0, None, op0=ALU.not_equal)
        pcount = ps_scan.tile([128, BC], F32, tag="ps_b", bufs=3)
        nc.tensor.matmul(pcount[:], lhsT=onesb[:], rhs=nz[:], start=True, stop=True)
        Lt = work.tile([128, BC], F32, tag="Lt")
        nc.vector.tensor_scalar_max(Lt[:], pcount[:], 1.0)
        qt = const.tile([128, BC], F32, tag="qt")
        c0b = _ap(c0c1[:, 0:1], [[2, 128], [0, BC]])
        nc.vector.scalar_tensor_tensor(
            qt[:], Lt[:], c0c1[:, 1:2], c0b, op0=ALU.mult, op1=ALU.add)

        ioi = big.tile([128, 16 * S], I32, tag="ioi")
        nc.gpsimd.iota(ioi[:], pattern=[[0, 16], [1, S]], base=0, channel_multiplier=0)
        iof = big.tile([128, 16 * S], F32, tag="iof")
        nc.vector.tensor_copy(iof[:], ioi[:])
        mk = big.tile([128, 16 * S], BF16, tag="mask")
        qv = _ap(qt[:, 0], [[BC, 128], [0, 2], [1, BC], [0, S]])
        nc.vector.tensor_tensor(
            mk[:].rearrange("p (ch b t) -> p ch b t", ch=2, b=BC),
            iof[:].rearrange("p (ch b t) -> p ch b t", ch=2, b=BC),
            qv, op=ALU.is_equal)

        # ---- LSTM scan: 128 sequential steps ----
        hist = big.tile([128, S * 16], BF16, tag="hist")
        hinit = const.tile([128, 16], BF16, tag="hinit")
        ctile = const.tile([128, 16], F32, tag="ctile")
        nc.vector.memset(hinit[:], 0)
        nc.vector.memset(ctile[:], 0)

        # gate-dim order (host pre-permuted): m-chunks 0,1=i  2,3=o  4,5=f  6,7=g
        # {f, g} matmuls run first into their own psum so sigmoid(f), tanh(g)
        # and f*c hide under the {i, o} matmuls; only sigmoid(i,o) + the c/h
        # tail sit on the recurrence cycle.
        for st in range(S):
            xg = xgh[st // 64]
            x0 = (st % 64) * 64
            hprev = hinit[:] if st == 0 else hist[:, (st - 1) * 16: st * 16]
            psa = ps_scan.tile([128, 32], F32, tag="ps_a", bufs=2)
            nc.tensor.matmul(psa[:], lhsT=idb[:], rhs=xg[:, x0 + 32: x0 + 64],
                             start=True, stop=False, skip_group_check=True)
            for k in range(2):
                for mc in (4, 5, 6, 7):
                    nc.tensor.matmul(
                        psa[:, (mc - 4) * 8:(mc - 3) * 8],
                        lhsT=whhT[:, k * G4 + mc * 128: k * G4 + (mc + 1) * 128],
                        rhs=hprev[:, k * 8:(k + 1) * 8],
                        start=False, stop=(k == 1 and mc == 7),
                        skip_group_check=True)
            psb = ps_scan.tile([128, 32], F32, tag="ps_b", bufs=3)
            nc.tensor.matmul(psb[:], lhsT=idb[:], rhs=xg[:, x0: x0 + 32],
                             start=True, stop=False, skip_group_check=True)
            for k in range(2):
                for mc in (0, 1, 2, 3):
                    nc.tensor.matmul(
                        psb[:, mc * 8:(mc + 1) * 8],
                        lhsT=whhT[:, k * G4 + mc * 128: k * G4 + (mc + 1) * 128],
                        rhs=hprev[:, k * 8:(k + 1) * 8],
                        start=False, stop=(k == 1 and mc == 3),
                        skip_group_check=True)
            sf = scan.tile([128, 16], F32, tag="sf")
            nc.scalar.activation(sf[:], psa[:, 0:16], AF.Sigmoid)
            gt = scan.tile([128, 16], F32, tag="gt")
            nc.scalar.activation(gt[:], psa[:, 16:32], AF.Tanh)
            sio = scan.tile([128, 32], F32, tag="sio")
            nc.scalar.activation(sio[:], psb[:], AF.Sigmoid)
            t2 = scan.tile([128, 16], F32, tag="t2")
            nc.vector.tensor_mul(t2[:], sf[:], ctile[:])
            t1 = scan.tile([128, 16], F32, tag="t1")
            nc.vector.tensor_mul(t1[:], sio[:, 0:16], gt[:])
            nc.vector.tensor_add(ctile[:], t1[:], t2[:])
            tau = scan.tile([128, 16], F32, tag="tau")
            nc.scalar.activation(tau[:], ctile[:], AF.Tanh)
            nc.vector.tensor_mul(hist[:, st * 16:(st + 1) * 16], sio[:, 16:32], tau[:])

        if PHASE < 4:
            finish_stub()
            return

        # ---- latch h at t = lengths-1 (fwd) / 128-lengths (rev step index) ----
        # split at t=120 so the bulk of the mask-multiply/reduce overlaps
        # the last scan steps (byte-range deps release hist[0:120] early)
        last = const.tile([128, 2 * BC], F32, tag="last")
        tmp = big.tile([128, 16 * S], F32, tag="latchtmp")
        tv = tmp[:].rearrange("p (c t) -> p c t", c=16)
        hv = hist[:].rearrange("p (t c) -> p c t", c=16)
        mv = mk[:].rearrange("p (c t) -> p c t", c=16)
        lastA = work.tile([128, 2 * BC], F32, tag="lastA")
        lastB = work.tile([128, 2 * BC], F32, tag="lastB")
        nc.vector.tensor_tensor(tv[:, :, 0:120], hv[:, :, 0:120],
                                mv[:, :, 0:120], op=ALU.mult)
        nc.vector.tensor_reduce(lastA[:], tv[:, :, 0:120],
                                axis=mybir.AxisListType.X, op=ALU.add)
        nc.vector.tensor_tensor(tv[:, :, 120:128], hv[:, :, 120:128],
                                mv[:, :, 120:128], op=ALU.mult)
        nc.vector.tensor_reduce(lastB[:], tv[:, :, 120:128],
                                axis=mybir.AxisListType.X, op=ALU.add)
        nc.vector.tensor_add(last[:], lastA[:], lastB[:])

        # ---- head ----
        def prelu(dst, src, alpha_ap):
            pos = work.tile(list(src.shape), F32, tag="prelu_pos")
            neg = work.tile(list(src.shape), F32, tag="prelu_neg")
            nc.vector.tensor_scalar_max(pos[:], src, 0.0)
            nc.vector.tensor_scalar_min(neg[:], src, 0.0)
            nc.vector.scalar_tensor_tensor(dst, neg[:], alpha_ap, pos[:],
                                           op0=ALU.mult, op1=ALU.add)

        pll = const.tile([128, 2 * BC], BF16, tag="pll")
        prelu(pll[:], last[:], al0[:, 0:1])
        plea = const.tile([128, BC], BF16, tag="plea")
        prelu(plea[:], embaT[:], al0[:, 0:1])

        # partial W1 product for own 8 sequences: px [128, 16] (m*8 + b)
        px = const.tile([128, 16], F32, tag="px")
        for m in range(2):
            pp = ps_scan.tile([128, BC], F32, tag="ps_b", bufs=3)
            for k in range(2):
                nc.tensor.matmul(
                    pp[:], lhsT=w1ta[:, k * 256 + m * 128: k * 256 + (m + 1) * 128],
                    rhs=pll[:, k * 8:(k + 1) * 8],
                    start=(k == 0), stop=False, skip_group_check=True)
            nc.tensor.matmul(pp[:], lhsT=w1tb[:, m * 128:(m + 1) * 128], rhs=plea[:],
                             start=False, stop=True, skip_group_check=True)
            nc.vector.tensor_copy(px[:, m * 8:(m + 1) * 8], pp[:])
        nc.sync.dma_start(d["partial"][:], px[:])
        nc.gpsimd.collective_compute(
            "AllReduce", ALU.add,
            replica_groups=[[0, 4], [1, 5], [2, 6], [3, 7]],
            ins=[d["partial"][:]], outs=[d["arshared"][:]])
        arx = const.tile([128, 16], F32, tag="arx")
        nc.sync.dma_start(arx[:], d["arshared"][:])

        x1 = const.tile([128, 16], BF16, tag="x1")
        for m in range(2):
            xb = work.tile([128, 8], F32, tag="xb")
            nc.vector.tensor_scalar(xb[:], arx[:, m * 8:(m + 1) * 8],
                                    b1c[:, m:m + 1], None, op0=ALU.add)
            prelu(x1[:, m * 8:(m + 1) * 8], xb[:], al1[:, 0:1])
        p2 = ps_scan.tile([64, BC], F32, tag="ps_b", bufs=3)
        for k in range(2):
            nc.tensor.matmul(p2[:], lhsT=w2t[:, k * 64:(k + 1) * 64],
                             rhs=x1[:, k * 8:(k + 1) * 8],
                             start=(k == 0), stop=(k == 1), skip_group_check=True)
        x2 = const.tile([64, BC], BF16, tag="x2")
        nc.scalar.activation(x2[:], p2[:], AF.Identity, bias=b2c[:, 0:1])
        pd = ps_scan.tile([1, BC], F32, tag="ps_b", bufs=3)
        nc.tensor.matmul(pd[:], lhsT=wdt[:], rhs=x2[:], start=True, stop=True,
                         skip_group_check=True)
        outs = const.tile([1, BC], F32, tag="outs")
        nc.scalar.activation(outs[:], pd[:], AF.Sigmoid, bias=bdc[:, 0:1])
        nc.sync.dma_start(d["out"][:], outs[:])

        if DBG:
            nc.sync.dma_start(d["dbg_q"][:], qt[:])
            nc.sync.dma_start(d["dbg_last"][:], last[:])
            nc.sync.dma_start(d["dbg_px"][:], px[:])
            nc.sync.dma_start(d["dbg_embT"][:], embT[:])
            nc.sync.dma_start(d["dbg_xg"][:], xg[:])
            nc.sync.dma_start(d["dbg_hist0"][:], hist[0][:])


_CACHE = {}

_IN_SPECS = [
    ("W_emb", (VOCAB, EMB), F32), ("idx", (128, BC), I32), ("idxa", (BC, 1), I32),
    ("whhT", (128, 2 * G4), WHH_DT), ("wihT", (128, G4), BF16), ("bvec", (128, 8), F32),
    ("identf", (128, 128), F32), ("identb", (128, 128), BF16), ("onesb", (128, 128), BF16),
    ("c0c1", (128, 2), F32), ("W1TA", (128, 512), BF16), ("W1TB", (128, 256), BF16),
    ("W2T", (128, 128), BF16), ("WdT", (64, 1), BF16), ("b1c", (128, 2), F32),
    ("b2c", (64, 1), F32), ("bdc", (1, 1), F32), ("al0", (128, 1), F32), ("al1", (128, 1), F32),
]


def _build():
    if "nc" in _CACHE:
        return _CACHE["nc"]
    nc = bacc.Bacc("TRN2", target_bir_lowering=False, debug=False, num_devices=NC)
    d = {}
    for name, shape, dt in _IN_SPECS:
        d[name] = nc.dram_tensor(name, shape, dt, kind="ExternalInput").ap()
    d["out"] = nc.dram_tensor("out", (1, BC), F32, kind="ExternalOutput").ap()
    d["partial"] = nc.dram_tensor("partial", (128, 16), F32, kind="Internal").ap()
    d["arshared"] = nc.dram_tensor("arshared", (128, 16), F32, kind="Internal").ap()
    if DBG:
        for nm, shape in [("dbg_q", (128, BC)), ("dbg_last", (128, 16)),
                          ("dbg_px", (128, 16))]:
            d[nm] = nc.dram_tensor(nm, shape, F32, kind="ExternalOutput").ap()
        for nm, shape in [("dbg_embT", (128, TOK)), ("dbg_xg", (128, S * 64)),
                          ("dbg_hist0", (128, S * 8))]:
            d[nm] = nc.dram_tensor(nm, shape, BF16, kind="ExternalOutput").ap()

    with tile.TileContext(nc) as tc:
        _emit(nc, tc, d)
    nc.compile()
    _CACHE["nc"] = nc
    return nc


def _prep_core_inputs(s, a, W_emb, w_ih_f, w_hh_f, b_f, w_ih_r, w_hh_r, b_r,
                      alpha0, alpha1, W1, b1, W2, b2, Wd, bd):
    """Host-side sharding / weight preprocessing -> list of 8 in_maps."""
    # gate-dim permutation: device order is [i, f, o, g]
    perm = np.r_[0:256, 768:1024, 256:512, 512:768]

    def eff(w_ih, w_hh, bb):
        wi = w_ih.astype(np.float64)[perm]
        wh = w_hh.astype(np.float64)[perm]
        be = bb.astype(np.float64)[perm]
        # whhT [128, 2*G4]: col k*G4 + gd  <-  w_hh.T[k*128+p, gd]
        whhT = np.empty((128, 2 * G4), np.float64)
        for k in range(2):
            whhT[:, k * G4:(k + 1) * G4] = wh[:, k * 128:(k + 1) * 128].T
        wihT = wi.T  # [128, 1024]
        bvec = be.reshape(8, 128).T.copy()  # bvec[p, mc] = be[mc*128+p]
        return (whhT.astype(WHH_NP), wihT.astype(BF16NP), bvec.astype(np.float32))

    whhT_f, wihT_f, bvec_f = eff(w_ih_f, w_hh_f, b_f)
    whhT_r, wihT_r, bvec_r = eff(w_ih_r, w_hh_r, b_r)

    # W1TA fwd: W1 cols 0:256 (h_f part); rev: W1 cols 256:512 (h_r part)
    def w1ta_for(col0):
        out = np.empty((128, 512), np.float32)
        for k in range(2):
            for m in range(2):
                blk = W1[m * 128:(m + 1) * 128, col0 + k * 128: col0 + (k + 1) * 128]
                out[:, k * 256 + m * 128: k * 256 + (m + 1) * 128] = blk.T
        return out.astype(BF16NP)

    w1ta_f = w1ta_for(0)
    w1ta_r = w1ta_for(256)
    w1tb_f = np.empty((128, 256), np.float32)
    for m in range(2):
        w1tb_f[:, m * 128:(m + 1) * 128] = W1[m * 128:(m + 1) * 128, 512:640].T
    w1tb_f = w1tb_f.astype(BF16NP)
    w1tb_r = np.zeros((128, 256), BF16NP)

    w2t = np.empty((128, 128), np.float32)
    for k in range(2):
        w2t[:, k * 64:(k + 1) * 64] = W2[:, k * 128:(k + 1) * 128].T
    w2t = w2t.astype(BF16NP)
    wdt = Wd.T.astype(BF16NP)                      # [64, 1]
    b1c = b1.reshape(2, 128).T.astype(np.float32)  # [128, 2]
    b2c = b2.reshape(64, 1).astype(np.float32)
    bdc = bd.reshape(1, 1).astype(np.float32)
    al0 = np.full((128, 1), float(np.asarray(alpha0).ravel()[0]), np.float32)
    al1 = np.full((128, 1), float(np.asarray(alpha1).ravel()[0]), np.float32)
    identf = np.eye(128, dtype=np.float32)
    identb = np.eye(128, dtype=np.float32).astype(BF16NP)
    onesb = np.ones((128, 128), np.float32).astype(BF16NP)
    W_emb32 = np.ascontiguousarray(W_emb.astype(np.float32))
    s = np.asarray(s).astype(np.int64)
    a = np.asarray(a).astype(np.int64)

    in_maps = []
    for c in range(NC):
        rev = c >= 4
        g = c % 4
        sg = s[g * 8:(g + 1) * 8]                  # [8, S]
        st = sg[:, ::-1] if rev else sg            # time order for this core
        # idx[p, m]: token n = m*128 + p ; (t, j) = (n//8, n%8)
        n = (np.arange(8)[None, :] * 128 + np.arange(128)[:, None])  # [128, 8]
        t_of = n // 8
        j_of = n % 8
        idxv = st[j_of, t_of].astype(np.int32)
        idxa = a[g * 8:(g + 1) * 8].astype(np.int32).reshape(BC, 1)
        c0 = 128.0 if rev else -1.0
        c1 = -1.0 if rev else 1.0
        c0c1 = np.tile(np.array([[c0, c1]], np.float32), (128, 1))
        in_maps.append({
            "W_emb": W_emb32, "idx": idxv, "idxa": idxa,
            "whhT": whhT_r if rev else whhT_f,
            "wihT": wihT_r if rev else wihT_f,
            "bvec": bvec_r if rev else bvec_f,
            "identf": identf, "identb": identb, "onesb": onesb,
            "c0c1": c0c1,
            "W1TA": w1ta_r if rev else w1ta_f,
            "W1TB": w1tb_r if rev else w1tb_f,
            "W2T": w2t, "WdT": wdt, "b1c": b1c, "b2c": b2c, "bdc": bdc,
            "al0": al0, "al1": al1,
        })
    return in_maps


def kernel(**inputs):
    inputs = {k: np.asarray(v) for k, v in inputs.items()}
    nc = _build()
    in_maps = _prep_core_inputs(**inputs)
    kwargs = {}
    if os.environ.get("KTRACE"):
        kwargs = dict(trace=True, trace_cores=list(range(NC)))
    res = run_bass_kernel_spmd(nc, in_maps, core_ids=list(range(NC)), **kwargs)
    _CACHE["last_results"] = res
    out = np.concatenate([res.results[g]["out"].reshape(BC) for g in range(4)])
    return out.reshape(B, 1).astype(np.float32)

